# revision 57
# baseline (speedup 1.0000x reference)
"""Trainium2 Bass kernel for nn_BaconAdditionReasoner (histogram_binning).

Math (per batch row b):
    P1 = soft_perm(W1), P2 = soft_perm(W2)          (host, 10x10)
    u = log(1 - p1 @ P1.T), v = log(1 - p2 @ P2.T)  (host prep, f16 upload)
    log1m[i,j] = log(1 - min(l1_i, l2_j)) == max(u_i, v_j)
    logprod[k] = sum_{i+j=k} max(u_i, v_j)
              = sum_{i+j=k} u_i  +  sum_{i+j=k} relu(v_j - u_i)
    e_k = exp(logprod_k)                            (device output)
    y_k = (1 - e_k) / sum_j (1 - e_j)               (host normalize; the
          denominator 19 - E is exactly the sum of the 19 numerators)

Device dataflow (pure data parallel over 8 cores, 32768 rows/core):
  Layout: features on partitions, batch on the free dim, 4 elements packed
  per column on 32-aligned 20-row bands (element (s,q,f) = 2048 s + 512 q
  + f lives in band q, column 512 s + f).  Supertile = 512 columns.

  Cost-model-driven choices: each matmul instruction costs out_cols x
  cycles_per_row serially on PE (f32=4, f32r/f16=1), elementwise ops cost
  free-cols per instruction on ACT(0.83ns/col)/DVE(1.04ns/col):
  - D: 4 f16 matmuls per supertile (20->110 pair diffs per band) into
    four 1-bank psum quarter tiles (q0/q1 double-buffered) so each
    D->relu->D WAR loop stays ~1.1us, under the 1704 ns PE cadence
  - A: 4 col-tiled f16 matmuls (110->32 anti-diagonal sums per band)
  - relu: ACT covers dp cols [0 : 512+AR1], DVE the rest (ACT 1690 /
    DVE 1624 ns per supertile); Exp on ACT; e values DMA out directly
  - f16 intermediates are safe: measured worst-case amplification ~10x
    on ~2.4e-4 rounding keeps y error ~2.5e-3, inside the 2e-2 gate
  - few big DMAs (HWDGE serializes at ~625ns per DMA); wk16 rides in the
    same DMA as the first uv chunk; one [128, 512] out-DMA per supertile
  - software pipeline lags (A -1, Exp/DMA -2) keep cross-engine deps a
    full supertile old; steady-state cadence is exactly the 8-matmul PE
    floor (1704 ns = 8 x 512 cols x 0.4167 ns)
"""

import numpy as np

# ---------------------------------------------------------------- constants
B = 262144
NCORES = 8
BC = B // NCORES            # 32768 rows per core
F = 512                     # batch columns per supertile
CH = 4                      # band count (32-aligned partition bands)
ROWS_ST = F * CH            # 2048 rows per supertile
NST = BC // ROWS_ST         # 16 supertiles per core
NCOLS = NST * F             # 8192 columns in pc / y
KD = 110                    # pair rows (100) + passthrough -u rows (10)

# wk16 column layout
WD0, WD1 = 0, 110           # D weights  [20, 110]
WA0, WA1 = 110, 142         # A weights  [110, 32]
WK16C = 142
AR1 = 336                   # cols of dp quarter 1 relu'd on ACT

IN_CHUNKS = (512, 1024, 1536, 2048, 2048, 1024)  # uv col splits (sum = NCOLS)
OUT_CHUNK = 4 * F                            # y cols per output chunk


def _soft_perm_np(W: np.ndarray) -> np.ndarray:
    W = W.astype(np.float32)
    lo = W.min(axis=1, keepdims=True)
    hi = W.max(axis=1, keepdims=True)
    Wn = (W - lo) / (hi - lo + np.float32(1e-8))
    return Wn / (Wn.sum(axis=1, keepdims=True) + np.float32(1e-8))




def _build_wk16() -> np.ndarray:
    wk = np.zeros((128, WK16C), dtype=np.float16)
    # --- D [20, 110]: col 10i+j gets v_j - u_i ; col 100+e passes -u_e.
    #     Replicated at each 32-row band: the ISA requires fmap and weights
    #     to start at the same SB partition.
    d = np.zeros((20, 110), dtype=np.float16)
    for i in range(10):
        for j in range(10):
            d[i, 10 * i + j] = -1.0
            d[10 + j, 10 * i + j] = 1.0
    for e in range(10):
        d[e, 100 + e] = -1.0
    for q in range(4):
        wk[32 * q : 32 * q + 20, WD0:WD1] = d
    # --- A [110, 32]: pair row 10i+j -> +1 at k=i+j ; row 100+e -> -1 for
    #     k in [e, e+9] (those rows hold -u, so -1 gives +u)
    for i in range(10):
        for j in range(10):
            wk[10 * i + j, WA0 + i + j] = 1.0
    for e in range(10):
        wk[100 + e, WA0 + e : WA0 + e + 10] = -1.0
    return wk


def _build_uv(uc: np.ndarray, vc: np.ndarray) -> np.ndarray:
    """u,v [BC,10] f32 -> uv [128, NCOLS] f16: band q on rows 32q+(0..9)=u,
    32q+(10..19)=v (32-aligned so fmap and D-weights share a partition
    base); col F*s+f = batch row ROWS_ST*s + F*q + f; junk rows zero."""
    u = uc.reshape(NST, CH, F, 10).transpose(1, 3, 0, 2).reshape(CH, 10, NCOLS)
    v = vc.reshape(NST, CH, F, 10).transpose(1, 3, 0, 2).reshape(CH, 10, NCOLS)
    out = np.zeros((128, NCOLS), dtype=np.float16)
    for q in range(CH):
        out[32 * q : 32 * q + 10] = u[q]
        out[32 * q + 10 : 32 * q + 20] = v[q]
    return out


def _unpack_y(yraw: np.ndarray) -> np.ndarray:
    """yraw [128, NCOLS] f16 (band q rows 32q+k, k<19: e_k = exp(logprod_k),
    k>=19 junk) -> y [BC, 19] f32 via the final normalize
    y = (1-e) / sum_k(1-e_k)  (the denominator 19-E is exactly that sum)."""
    t = yraw.astype(np.float32).reshape(4, 32, NST, F).transpose(2, 0, 3, 1)
    t = 1.0 - t.reshape(BC, 32)[:, :19]
    return t / t.sum(axis=1, keepdims=True)


def _patch_act_tables():
    """No-op: with only Exp and Relu used on ACT, the greedy chooser picks
    the single exp_and_others set by itself (no table ping-pong)."""
    return


ROLES = {}


def _rec(role, obj):
    try:
        ROLES[obj.ins.name] = role
    except Exception:
        pass
    return obj


def build_bass():
    import concourse.bacc as bacc
    import concourse.tile as tile
    from concourse import mybir

    _patch_act_tables()
    f32 = mybir.dt.float32
    f32r = mybir.dt.float32r
    f16 = mybir.dt.float16
    AF = mybir.ActivationFunctionType
    ALU = mybir.AluOpType

    nc = bacc.Bacc("TRN2", target_bir_lowering=False)

    # head = wk16 columns + the first uv chunk, fetched in ONE DMA so the
    # first D matmul starts ~600ns earlier (HWDGE serializes at 625ns/DMA)
    hd_d = nc.dram_tensor(
        "head", [128, WK16C + IN_CHUNKS[0]], f16, kind="ExternalInput"
    )
    uv_d = nc.dram_tensor(
        "uvp", [128, NCOLS - IN_CHUNKS[0]], f16, kind="ExternalInput"
    )
    y_d = nc.dram_tensor("yraw", [128, NCOLS], f16, kind="ExternalOutput")

    with tile.TileContext(nc) as tc:
        with (
            tc.tile_pool(name="singles", bufs=1) as singles,
            tc.tile_pool(name="kt", bufs=3) as kt_p,
            tc.tile_pool(name="kt2", bufs=3) as kt2_p,
            tc.tile_pool(name="ep", bufs=4) as ep_p,
            tc.tile_pool(name="psD0", bufs=1, space="PSUM") as psD0,
            tc.tile_pool(name="psD1", bufs=1, space="PSUM") as psD1,
            tc.tile_pool(name="psD2", bufs=1, space="PSUM") as psD2,
            tc.tile_pool(name="psD3", bufs=2, space="PSUM") as psD3,
            tc.tile_pool(name="psZ", bufs=2, space="PSUM") as psZ,
        ):
            head = singles.tile([128, WK16C + IN_CHUNKS[0]], f16)
            nc.sync.dma_start(head[:, :], hd_d[:, :])
            wk = head  # wk16 columns live at head[:, 0:WK16C]

            packs = [(0, IN_CHUNKS[0], None)]
            c0 = IN_CHUNKS[0]
            for w in IN_CHUNKS[1:]:
                p = singles.tile([128, w], f16, name=f"pk{c0}")
                nc.sync.dma_start(p[:, :], uv_d[:, c0 - IN_CHUNKS[0] : c0 - IN_CHUNKS[0] + w])
                packs.append((c0, w, p))
                c0 += w

            def uv_slice(col0, r0, r1):
                if col0 < IN_CHUNKS[0]:
                    return head[r0:r1, WK16C + col0 : WK16C + col0 + F]
                for c0, w, p in packs:
                    if p is not None and c0 <= col0 < c0 + w:
                        return p[r0:r1, col0 - c0 : col0 - c0 + F]
                raise AssertionError(col0)

            # Software pipeline, one iteration per supertile `it`:
            #   PE:  D(it)x4, A(it-1)x4               (8 x 213 ns)
            #   ACT: Exp(it-2), relu[0:1024](it)
            #   DVE: relu[1024:2048](it)
            #   SP:  DMA of ep(it-2) straight from the Exp output tile
            # Lags keep every cross-engine dependency >= 1 supertile old;
            # the D(s+1)-overwrites-dp WAR loops stay under the 1704 ns PE
            # cadence (dp1, under the 1192ns DVE relu, is double-buffered).
            kts, eps = {}, {}
            for it in range(NST + 3):
                # ---- D(it) + relu(it)
                if it < NST:
                    off = F * it
                    ktq = [
                        kt_p.tile([KD, F], f16, name=f"ktq{h}")
                        for h in range(2)
                    ] + [
                        kt2_p.tile([KD, F], f16, name=f"ktq{h}")
                        for h in range(2, 4)
                    ]
                    dps = [
                        psD0.tile([KD, F], f32, name="dp0"),
                        psD1.tile([KD, F], f32, name="dp1"),
                        psD2.tile([KD, F], f32, name="dp2"),
                        psD3.tile([KD, F], f32, name="dp3"),
                    ]
                    for g in (2, 3, 0, 1):
                        _rec(f"D{g}({it})", nc.tensor.matmul(
                            dps[g][0:KD, :],
                            wk[32 * g : 32 * g + 20, WD0:WD1],
                            uv_slice(off, 32 * g, 32 * g + 20),
                            start=True, stop=True,
                            tile_position=(32 * g, 0),
                        ))
                    # relu: ACT covers [0 : F+AR1], DVE the rest -- both
                    # stay under the 1704ns PE cadence, and per-quarter
                    # psum tiles keep each D->relu->D WAR loop short
                    _rec(f"reluA0({it})", nc.scalar.activation(
                        ktq[0][0:KD, :], dps[0][0:KD, :], AF.Relu
                    ))
                    # last supertile: ACT takes all of q1 (it has drain
                    # slack; the DVE V1 piece otherwise runs last and
                    # stalls A1)
                    a1 = F if it == NST - 1 else AR1
                    _rec(f"reluA1({it})", nc.scalar.activation(
                        ktq[1][0:KD, 0:a1], dps[1][0:KD, 0:a1], AF.Relu
                    ))
                    if a1 < F:
                        _rec(f"reluV1({it})", nc.vector.tensor_scalar(
                            ktq[1][0:KD, a1:F], dps[1][0:KD, a1:F],
                            0.0, None, op0=ALU.max,
                        ))
                    _rec(f"reluV2({it})", nc.vector.tensor_scalar(
                        ktq[2][0:KD, :], dps[2][0:KD, :],
                        0.0, None, op0=ALU.max,
                    ))
                    _rec(f"reluV3({it})", nc.vector.tensor_scalar(
                        ktq[3][0:KD, :], dps[3][0:KD, :],
                        0.0, None, op0=ALU.max,
                    ))
                    kts[it] = ktq
                # ---- A(it-1): 4 col-tiled f16 matmuls, band q <- quarter q
                sA = it - 1
                if 0 <= sA < NST:
                    ktq = kts.pop(sA)
                    zz = psZ.tile([128, F], f32)
                    # order 0,2,3,1: the DVE relu pieces complete V2,V3,V1
                    # (scheduler frees single-buffered q2/q3 first), and PE
                    # streams execute in order -- A1 last avoids a drain
                    # stall behind its late quarter
                    for g in (0, 2, 3, 1):
                        _rec(f"A{g}({sA})", nc.tensor.matmul(
                            zz[32 * g : 32 * g + 32, :],
                            wk[0:KD, WA0:WA1],
                            ktq[g][0:KD, :],
                            start=True, stop=True,
                            tile_position=(0, 32 * g),
                        ))
                    eps[sA] = zz
                # ---- Exp(it-2), then DMA the e values straight out
                sE = it - 2
                if 0 <= sE < NST:
                    zz = eps.pop(sE)
                    ep = ep_p.tile([128, F], f16)
                    _rec(f"Exp({sE})", nc.scalar.activation(
                        ep[:, :], zz[:, :], AF.Exp
                    ))
                    oc0 = F * sE
                    nc.sync.dma_start(y_d[:, oc0 : oc0 + F], ep[:, :])
    nc.compile()
    return nc


_NC_CACHE = None


def kernel(p1, p2, W1, W2):
    global _NC_CACHE
    from concourse.bass_utils import run_bass_kernel_spmd

    P1n = _soft_perm_np(np.asarray(W1))
    P2n = _soft_perm_np(np.asarray(W2))
    wk16 = _build_wk16()
    p1 = np.asarray(p1, dtype=np.float32)
    p2 = np.asarray(p2, dtype=np.float32)
    u = np.log1p(-(p1 @ P1n.T)).astype(np.float32)
    v = np.log1p(-(p2 @ P2n.T)).astype(np.float32)

    in_maps = []
    c0 = IN_CHUNKS[0]
    for c in range(NCORES):
        sl = slice(c * BC, (c + 1) * BC)
        uvp = _build_uv(u[sl], v[sl])
        head = np.concatenate([wk16, uvp[:, :c0]], axis=1)
        in_maps.append(
            {"head": np.ascontiguousarray(head), "uvp": uvp[:, c0:].copy()}
        )

    if _NC_CACHE is None:
        _NC_CACHE = build_bass()
    res = run_bass_kernel_spmd(_NC_CACHE, in_maps, core_ids=list(range(NCORES)))
    out = np.concatenate(
        [_unpack_y(res.results[c]["yraw"]) for c in range(NCORES)], axis=0
    )
    return out


# revision 58
# speedup vs baseline: 1.0068x; 1.0068x over previous
"""Trainium2 Bass kernel for nn_BaconAdditionReasoner (histogram_binning).

Math (per batch row b):
    P1 = soft_perm(W1), P2 = soft_perm(W2)          (host, 10x10)
    u = log(1 - p1 @ P1.T), v = log(1 - p2 @ P2.T)  (host prep, f16 upload)
    log1m[i,j] = log(1 - min(l1_i, l2_j)) == max(u_i, v_j)
    logprod[k] = sum_{i+j=k} max(u_i, v_j)
              = sum_{i+j=k} u_i  +  sum_{i+j=k} relu(v_j - u_i)
    e_k = exp(logprod_k)                            (device output)
    y_k = (1 - e_k) / sum_j (1 - e_j)               (host normalize; the
          denominator 19 - E is exactly the sum of the 19 numerators)

Device dataflow (pure data parallel over 8 cores, 32768 rows/core):
  Layout: features on partitions, batch on the free dim, 4 elements packed
  per column on 32-aligned 20-row bands (element (s,q,f) = 2048 s + 512 q
  + f lives in band q, column 512 s + f).  Supertile = 512 columns.

  Cost-model-driven choices: each matmul instruction costs out_cols x
  cycles_per_row serially on PE (f32=4, f32r/f16=1), elementwise ops cost
  free-cols per instruction on ACT(0.83ns/col)/DVE(1.04ns/col):
  - D: 4 f16 matmuls per supertile (20->110 pair diffs per band) into
    four 1-bank psum quarter tiles (q0/q1 double-buffered) so each
    D->relu->D WAR loop stays ~1.1us, under the 1704 ns PE cadence
  - A: 4 col-tiled f16 matmuls (110->32 anti-diagonal sums per band)
  - relu: ACT covers dp cols [0 : 512+AR1], DVE the rest (ACT 1690 /
    DVE 1624 ns per supertile); Exp on ACT; e values DMA out directly
  - f16 intermediates are safe: measured worst-case amplification ~10x
    on ~2.4e-4 rounding keeps y error ~2.5e-3, inside the 2e-2 gate
  - few big DMAs (HWDGE serializes at ~625ns per DMA); wk16 rides in the
    same DMA as the first uv chunk; one [128, 512] out-DMA per supertile
  - software pipeline lags (A -1, Exp/DMA -2) keep cross-engine deps a
    full supertile old; steady-state cadence is exactly the 8-matmul PE
    floor (1704 ns = 8 x 512 cols x 0.4167 ns)
"""

import numpy as np

# ---------------------------------------------------------------- constants
B = 262144
NCORES = 8
BC = B // NCORES            # 32768 rows per core
F = 512                     # batch columns per supertile
CH = 4                      # band count (32-aligned partition bands)
ROWS_ST = F * CH            # 2048 rows per supertile
NST = BC // ROWS_ST         # 16 supertiles per core
NCOLS = NST * F             # 8192 columns in pc / y
KD = 110                    # pair rows (100) + passthrough -u rows (10)

# wk16 column layout
WD0, WD1 = 0, 110           # D weights  [20, 110]
WA0, WA1 = 110, 142         # A weights  [110, 32]
WK16C = 142
AR1 = 336                   # cols of dp quarter 1 relu'd on ACT

IN_CHUNKS = (512, 1024, 1536, 2048, 2048, 1024)  # uv col splits (sum = NCOLS)
OUT_CHUNK = 4 * F                            # y cols per output chunk


def _soft_perm_np(W: np.ndarray) -> np.ndarray:
    W = W.astype(np.float32)
    lo = W.min(axis=1, keepdims=True)
    hi = W.max(axis=1, keepdims=True)
    Wn = (W - lo) / (hi - lo + np.float32(1e-8))
    return Wn / (Wn.sum(axis=1, keepdims=True) + np.float32(1e-8))




def _build_wk16() -> np.ndarray:
    wk = np.zeros((128, WK16C), dtype=np.float16)
    # --- D [20, 110]: col 10i+j gets v_j - u_i ; col 100+e passes -u_e.
    #     Replicated at each 32-row band: the ISA requires fmap and weights
    #     to start at the same SB partition.
    d = np.zeros((20, 110), dtype=np.float16)
    for i in range(10):
        for j in range(10):
            d[i, 10 * i + j] = -1.0
            d[10 + j, 10 * i + j] = 1.0
    for e in range(10):
        d[e, 100 + e] = -1.0
    for q in range(4):
        wk[32 * q : 32 * q + 20, WD0:WD1] = d
    # --- A [110, 32]: pair row 10i+j -> +1 at k=i+j ; row 100+e -> -1 for
    #     k in [e, e+9] (those rows hold -u, so -1 gives +u)
    for i in range(10):
        for j in range(10):
            wk[10 * i + j, WA0 + i + j] = 1.0
    for e in range(10):
        wk[100 + e, WA0 + e : WA0 + e + 10] = -1.0
    return wk


def _build_uv(uc: np.ndarray, vc: np.ndarray) -> np.ndarray:
    """u,v [BC,10] f32 -> uv [128, NCOLS] f16: band q on rows 32q+(0..9)=u,
    32q+(10..19)=v (32-aligned so fmap and D-weights share a partition
    base); col F*s+f = batch row ROWS_ST*s + F*q + f; junk rows zero."""
    u = uc.reshape(NST, CH, F, 10).transpose(1, 3, 0, 2).reshape(CH, 10, NCOLS)
    v = vc.reshape(NST, CH, F, 10).transpose(1, 3, 0, 2).reshape(CH, 10, NCOLS)
    out = np.zeros((128, NCOLS), dtype=np.float16)
    for q in range(CH):
        out[32 * q : 32 * q + 10] = u[q]
        out[32 * q + 10 : 32 * q + 20] = v[q]
    return out


def _unpack_y(yraw: np.ndarray) -> np.ndarray:
    """yraw [128, NCOLS] f16 (band q rows 32q+k, k<19: e_k = exp(logprod_k),
    k>=19 junk) -> y [BC, 19] f32 via the final normalize
    y = (1-e) / sum_k(1-e_k)  (the denominator 19-E is exactly that sum)."""
    t = yraw.astype(np.float32).reshape(4, 32, NST, F).transpose(2, 0, 3, 1)
    t = 1.0 - t.reshape(BC, 32)[:, :19]
    return t / t.sum(axis=1, keepdims=True)


def _patch_act_tables():
    """No-op: with only Exp and Relu used on ACT, the greedy chooser picks
    the single exp_and_others set by itself (no table ping-pong)."""
    return


ROLES = {}


def _rec(role, obj):
    try:
        ROLES[obj.ins.name] = role
    except Exception:
        pass
    return obj


def build_bass():
    import concourse.bacc as bacc
    import concourse.tile as tile
    from concourse import mybir

    _patch_act_tables()
    f32 = mybir.dt.float32
    f32r = mybir.dt.float32r
    f16 = mybir.dt.float16
    AF = mybir.ActivationFunctionType
    ALU = mybir.AluOpType

    nc = bacc.Bacc("TRN2", target_bir_lowering=False)

    # head = wk16 columns + the first uv chunk, fetched in ONE DMA so the
    # first D matmul starts ~600ns earlier (HWDGE serializes at 625ns/DMA)
    hd_d = nc.dram_tensor(
        "head", [128, WK16C + IN_CHUNKS[0]], f16, kind="ExternalInput"
    )
    uv_d = nc.dram_tensor(
        "uvp", [128, NCOLS - IN_CHUNKS[0]], f16, kind="ExternalInput"
    )
    y_d = nc.dram_tensor("yraw", [128, NCOLS], f16, kind="ExternalOutput")

    with tile.TileContext(nc) as tc:
        with (
            tc.tile_pool(name="singles", bufs=1) as singles,
            tc.tile_pool(name="kt", bufs=3) as kt_p,
            tc.tile_pool(name="kt2", bufs=3) as kt2_p,
            tc.tile_pool(name="ep", bufs=4) as ep_p,
            tc.tile_pool(name="psD0", bufs=1, space="PSUM") as psD0,
            tc.tile_pool(name="psD1", bufs=1, space="PSUM") as psD1,
            tc.tile_pool(name="psD2", bufs=1, space="PSUM") as psD2,
            tc.tile_pool(name="psD3", bufs=2, space="PSUM") as psD3,
            tc.tile_pool(name="psZ", bufs=2, space="PSUM") as psZ,
        ):
            head = singles.tile([128, WK16C + IN_CHUNKS[0]], f16)
            nc.sync.dma_start(head[:, :], hd_d[:, :])
            wk = head  # wk16 columns live at head[:, 0:WK16C]

            packs = [(0, IN_CHUNKS[0], None)]
            c0 = IN_CHUNKS[0]
            for w in IN_CHUNKS[1:]:
                p = singles.tile([128, w], f16, name=f"pk{c0}")
                nc.sync.dma_start(p[:, :], uv_d[:, c0 - IN_CHUNKS[0] : c0 - IN_CHUNKS[0] + w])
                packs.append((c0, w, p))
                c0 += w

            def uv_slice(col0, r0, r1):
                if col0 < IN_CHUNKS[0]:
                    return head[r0:r1, WK16C + col0 : WK16C + col0 + F]
                for c0, w, p in packs:
                    if p is not None and c0 <= col0 < c0 + w:
                        return p[r0:r1, col0 - c0 : col0 - c0 + F]
                raise AssertionError(col0)

            # Software pipeline, one iteration per supertile `it`:
            #   PE:  D(it)x4, A(it-1)x4               (8 x 213 ns)
            #   ACT: Exp(it-2), relu[0:1024](it)
            #   DVE: relu[1024:2048](it)
            #   SP:  DMA of ep(it-2) straight from the Exp output tile
            # Lags keep every cross-engine dependency >= 1 supertile old;
            # the D(s+1)-overwrites-dp WAR loops stay under the 1704 ns PE
            # cadence (dp1, under the 1192ns DVE relu, is double-buffered).
            kts, eps = {}, {}
            for it in range(NST + 3):
                # ---- D(it) + relu(it)
                if it < NST:
                    off = F * it
                    ktq = [
                        kt_p.tile([KD, F], f16, name=f"ktq{h}")
                        for h in range(2)
                    ] + [
                        kt2_p.tile([KD, F], f16, name=f"ktq{h}")
                        for h in range(2, 4)
                    ]
                    dps = [
                        psD0.tile([KD, F], f32, name="dp0"),
                        psD1.tile([KD, F], f32, name="dp1"),
                        psD2.tile([KD, F], f32, name="dp2"),
                        psD3.tile([KD, F], f32, name="dp3"),
                    ]
                    for g in (0, 2, 3, 1):
                        _rec(f"D{g}({it})", nc.tensor.matmul(
                            dps[g][0:KD, :],
                            wk[32 * g : 32 * g + 20, WD0:WD1],
                            uv_slice(off, 32 * g, 32 * g + 20),
                            start=True, stop=True,
                            tile_position=(32 * g, 0),
                        ))
                    # relu: ACT covers [0 : F+AR1], DVE the rest -- both
                    # stay under the 1704ns PE cadence, and per-quarter
                    # psum tiles keep each D->relu->D WAR loop short
                    _rec(f"reluA0({it})", nc.scalar.activation(
                        ktq[0][0:KD, :], dps[0][0:KD, :], AF.Relu
                    ))
                    # last supertile: ACT takes all of q1 (it has drain
                    # slack; the DVE V1 piece otherwise runs last and
                    # stalls A1)
                    a1 = F if it == NST - 1 else AR1
                    _rec(f"reluA1({it})", nc.scalar.activation(
                        ktq[1][0:KD, 0:a1], dps[1][0:KD, 0:a1], AF.Relu
                    ))
                    if a1 < F:
                        _rec(f"reluV1({it})", nc.vector.tensor_scalar(
                            ktq[1][0:KD, a1:F], dps[1][0:KD, a1:F],
                            0.0, None, op0=ALU.max,
                        ))
                    _rec(f"reluV2({it})", nc.vector.tensor_scalar(
                        ktq[2][0:KD, :], dps[2][0:KD, :],
                        0.0, None, op0=ALU.max,
                    ))
                    _rec(f"reluV3({it})", nc.vector.tensor_scalar(
                        ktq[3][0:KD, :], dps[3][0:KD, :],
                        0.0, None, op0=ALU.max,
                    ))
                    kts[it] = ktq
                # ---- A(it-1): 4 col-tiled f16 matmuls, band q <- quarter q
                sA = it - 1
                if 0 <= sA < NST:
                    ktq = kts.pop(sA)
                    zz = psZ.tile([128, F], f32)
                    # order 0,2,3,1: the DVE relu pieces complete V2,V3,V1
                    # (scheduler frees single-buffered q2/q3 first), and PE
                    # streams execute in order -- A1 last avoids a drain
                    # stall behind its late quarter
                    for g in (0, 2, 3, 1):
                        _rec(f"A{g}({sA})", nc.tensor.matmul(
                            zz[32 * g : 32 * g + 32, :],
                            wk[0:KD, WA0:WA1],
                            ktq[g][0:KD, :],
                            start=True, stop=True,
                            tile_position=(0, 32 * g),
                        ))
                    eps[sA] = zz
                # ---- Exp(it-2), then DMA the e values straight out
                sE = it - 2
                if 0 <= sE < NST:
                    zz = eps.pop(sE)
                    ep = ep_p.tile([128, F], f16)
                    _rec(f"Exp({sE})", nc.scalar.activation(
                        ep[:, :], zz[:, :], AF.Exp
                    ))
                    oc0 = F * sE
                    nc.sync.dma_start(y_d[:, oc0 : oc0 + F], ep[:, :])
    nc.compile()
    return nc


_NC_CACHE = None


def kernel(p1, p2, W1, W2):
    global _NC_CACHE
    from concourse.bass_utils import run_bass_kernel_spmd

    P1n = _soft_perm_np(np.asarray(W1))
    P2n = _soft_perm_np(np.asarray(W2))
    wk16 = _build_wk16()
    p1 = np.asarray(p1, dtype=np.float32)
    p2 = np.asarray(p2, dtype=np.float32)
    u = np.log1p(-(p1 @ P1n.T)).astype(np.float32)
    v = np.log1p(-(p2 @ P2n.T)).astype(np.float32)

    in_maps = []
    c0 = IN_CHUNKS[0]
    for c in range(NCORES):
        sl = slice(c * BC, (c + 1) * BC)
        uvp = _build_uv(u[sl], v[sl])
        head = np.concatenate([wk16, uvp[:, :c0]], axis=1)
        in_maps.append(
            {"head": np.ascontiguousarray(head), "uvp": uvp[:, c0:].copy()}
        )

    if _NC_CACHE is None:
        _NC_CACHE = build_bass()
    res = run_bass_kernel_spmd(_NC_CACHE, in_maps, core_ids=list(range(NCORES)))
    out = np.concatenate(
        [_unpack_y(res.results[c]["yraw"]) for c in range(NCORES)], axis=0
    )
    return out


# revision 64
# speedup vs baseline: 1.0211x; 1.0142x over previous
"""Trainium2 Bass kernel for nn_BaconAdditionReasoner (histogram_binning).

Math (per batch row b):
    P1 = soft_perm(W1), P2 = soft_perm(W2)          (host, 10x10)
    u = log(1 - p1 @ P1.T), v = log(1 - p2 @ P2.T)  (host prep, f16 upload)
    log1m[i,j] = log(1 - min(l1_i, l2_j)) == max(u_i, v_j)
    logprod[k] = sum_{i+j=k} max(u_i, v_j)
              = sum_{i+j=k} u_i  +  sum_{i+j=k} relu(v_j - u_i)
    e_k = exp(logprod_k)                            (device output)
    y_k = (1 - e_k) / sum_j (1 - e_j)               (host normalize; the
          denominator 19 - E is exactly the sum of the 19 numerators)

Device dataflow (pure data parallel over 8 cores, 32768 rows/core):
  Layout: features on partitions, batch on the free dim, 4 elements packed
  per column on 32-aligned 20-row bands (element (s,q,f) = 2048 s + 512 q
  + f lives in band q, column 512 s + f).  Supertile = 512 columns.

  Cost-model-driven choices: each matmul instruction costs out_cols x
  cycles_per_row serially on PE (f32=4, f32r/f16=1), elementwise ops cost
  free-cols per instruction on ACT(0.83ns/col)/DVE(1.04ns/col):
  - D: 4 f16 matmuls per supertile (20->110 pair diffs per band) into
    four 1-bank psum quarter tiles (q0/q1 double-buffered) so each
    D->relu->D WAR loop stays ~1.1us, under the 1704 ns PE cadence
  - A: 4 col-tiled f16 matmuls (110->32 anti-diagonal sums per band)
  - relu: ACT covers dp cols [0 : 512+AR1], DVE the rest (ACT 1690 /
    DVE 1624 ns per supertile); Exp on ACT; e values DMA out directly
  - f16 intermediates are safe: measured worst-case amplification ~10x
    on ~2.4e-4 rounding keeps y error ~2.5e-3, inside the 2e-2 gate
  - few big DMAs (HWDGE serializes at ~625ns per DMA); wk16 rides in the
    same DMA as the first uv chunk; one [128, 512] out-DMA per supertile
  - software pipeline lags (A -1, Exp/DMA -2) keep cross-engine deps a
    full supertile old; steady-state cadence is exactly the 8-matmul PE
    floor (1704 ns = 8 x 512 cols x 0.4167 ns)
"""

import numpy as np

# ---------------------------------------------------------------- constants
B = 262144
NCORES = 8
BC = B // NCORES            # 32768 rows per core
F = 512                     # batch columns per supertile
CH = 4                      # band count (32-aligned partition bands)
ROWS_ST = F * CH            # 2048 rows per supertile
NST = BC // ROWS_ST         # 16 supertiles per core
NCOLS = NST * F             # 8192 columns in pc / y
KD = 110                    # pair rows (100) + passthrough -u rows (10)

# wk16 column layout
WD0, WD1 = 0, 110           # D weights  [20, 110]
WA0, WA1 = 110, 142         # A weights  [110, 32]
WK16C = 142
AR1 = 304                   # cols of dp quarter 1 relu'd on ACT

IN_CHUNKS = (512, 1024, 1536, 2048, 2048, 1024)  # uv col splits (sum = NCOLS)
OUT_CHUNK = 4 * F                            # y cols per output chunk


def _soft_perm_np(W: np.ndarray) -> np.ndarray:
    W = W.astype(np.float32)
    lo = W.min(axis=1, keepdims=True)
    hi = W.max(axis=1, keepdims=True)
    Wn = (W - lo) / (hi - lo + np.float32(1e-8))
    return Wn / (Wn.sum(axis=1, keepdims=True) + np.float32(1e-8))




def _build_wk16() -> np.ndarray:
    wk = np.zeros((128, WK16C), dtype=np.float16)
    # --- D [20, 110]: col 10i+j gets v_j - u_i ; col 100+e passes -u_e.
    #     Replicated at each 32-row band: the ISA requires fmap and weights
    #     to start at the same SB partition.
    d = np.zeros((20, 110), dtype=np.float16)
    for i in range(10):
        for j in range(10):
            d[i, 10 * i + j] = -1.0
            d[10 + j, 10 * i + j] = 1.0
    for e in range(10):
        d[e, 100 + e] = -1.0
    for q in range(4):
        wk[32 * q : 32 * q + 20, WD0:WD1] = d
    # --- A [110, 32]: pair row 10i+j -> +1 at k=i+j ; row 100+e -> -1 for
    #     k in [e, e+9] (those rows hold -u, so -1 gives +u)
    for i in range(10):
        for j in range(10):
            wk[10 * i + j, WA0 + i + j] = 1.0
    for e in range(10):
        wk[100 + e, WA0 + e : WA0 + e + 10] = -1.0
    return wk


def _build_uv(uc: np.ndarray, vc: np.ndarray) -> np.ndarray:
    """u,v [BC,10] f32 -> uv [128, NCOLS] f16: band q on rows 32q+(0..9)=u,
    32q+(10..19)=v (32-aligned so fmap and D-weights share a partition
    base); col F*s+f = batch row ROWS_ST*s + F*q + f; junk rows zero."""
    u = uc.reshape(NST, CH, F, 10).transpose(1, 3, 0, 2).reshape(CH, 10, NCOLS)
    v = vc.reshape(NST, CH, F, 10).transpose(1, 3, 0, 2).reshape(CH, 10, NCOLS)
    out = np.zeros((128, NCOLS), dtype=np.float16)
    for q in range(CH):
        out[32 * q : 32 * q + 10] = u[q]
        out[32 * q + 10 : 32 * q + 20] = v[q]
    return out


def _unpack_y(yraw: np.ndarray) -> np.ndarray:
    """yraw [128, NCOLS] f16 (band q rows 32q+k, k<19: e_k = exp(logprod_k),
    k>=19 junk) -> y [BC, 19] f32 via the final normalize
    y = (1-e) / sum_k(1-e_k)  (the denominator 19-E is exactly that sum)."""
    t = yraw.astype(np.float32).reshape(4, 32, NST, F).transpose(2, 0, 3, 1)
    t = 1.0 - t.reshape(BC, 32)[:, :19]
    return t / t.sum(axis=1, keepdims=True)


def _patch_act_tables():
    """No-op: with only Exp and Relu used on ACT, the greedy chooser picks
    the single exp_and_others set by itself (no table ping-pong)."""
    return


ROLES = {}


def _rec(role, obj):
    try:
        ROLES[obj.ins.name] = role
    except Exception:
        pass
    return obj


def build_bass():
    import concourse.bacc as bacc
    import concourse.tile as tile
    from concourse import mybir

    _patch_act_tables()
    f32 = mybir.dt.float32
    f32r = mybir.dt.float32r
    f16 = mybir.dt.float16
    AF = mybir.ActivationFunctionType
    ALU = mybir.AluOpType

    nc = bacc.Bacc("TRN2", target_bir_lowering=False)

    # head = wk16 columns + the first uv chunk, fetched in ONE DMA so the
    # first D matmul starts ~600ns earlier (HWDGE serializes at 625ns/DMA)
    hd_d = nc.dram_tensor(
        "head", [128, WK16C + IN_CHUNKS[0]], f16, kind="ExternalInput"
    )
    uv_d = nc.dram_tensor(
        "uvp", [128, NCOLS - IN_CHUNKS[0]], f16, kind="ExternalInput"
    )
    y_d = nc.dram_tensor("yraw", [128, NCOLS], f16, kind="ExternalOutput")

    with tile.TileContext(nc) as tc:
        with (
            tc.tile_pool(name="singles", bufs=1) as singles,
            tc.tile_pool(name="kt", bufs=3) as kt_p,
            tc.tile_pool(name="kt2", bufs=3) as kt2_p,
            tc.tile_pool(name="ep", bufs=4) as ep_p,
            tc.tile_pool(name="psD0", bufs=1, space="PSUM") as psD0,
            tc.tile_pool(name="psD1", bufs=1, space="PSUM") as psD1,
            tc.tile_pool(name="psD2", bufs=1, space="PSUM") as psD2,
            tc.tile_pool(name="psD3", bufs=2, space="PSUM") as psD3,
            tc.tile_pool(name="psZ", bufs=2, space="PSUM") as psZ,
        ):
            head = singles.tile([128, WK16C + IN_CHUNKS[0]], f16)
            nc.sync.dma_start(head[:, :], hd_d[:, :])
            wk = head  # wk16 columns live at head[:, 0:WK16C]

            packs = [(0, IN_CHUNKS[0], None)]
            c0 = IN_CHUNKS[0]
            for w in IN_CHUNKS[1:]:
                p = singles.tile([128, w], f16, name=f"pk{c0}")
                nc.sync.dma_start(p[:, :], uv_d[:, c0 - IN_CHUNKS[0] : c0 - IN_CHUNKS[0] + w])
                packs.append((c0, w, p))
                c0 += w

            def uv_slice(col0, r0, r1):
                if col0 < IN_CHUNKS[0]:
                    return head[r0:r1, WK16C + col0 : WK16C + col0 + F]
                for c0, w, p in packs:
                    if p is not None and c0 <= col0 < c0 + w:
                        return p[r0:r1, col0 - c0 : col0 - c0 + F]
                raise AssertionError(col0)

            # Software pipeline, one iteration per supertile `it`:
            #   PE:  D(it)x4, A(it-1)x4               (8 x 213 ns)
            #   ACT: Exp(it-2), relu[0:1024](it)
            #   DVE: relu[1024:2048](it)
            #   SP:  DMA of ep(it-2) straight from the Exp output tile
            # Lags keep every cross-engine dependency >= 1 supertile old;
            # the D(s+1)-overwrites-dp WAR loops stay under the 1704 ns PE
            # cadence (dp1, under the 1192ns DVE relu, is double-buffered).
            kts, eps = {}, {}
            for it in range(NST + 3):
                # ---- D(it) + relu(it)
                if it < NST:
                    off = F * it
                    ktq = [
                        kt_p.tile([KD, F], f16, name=f"ktq{h}")
                        for h in range(2)
                    ] + [
                        kt2_p.tile([KD, F], f16, name=f"ktq{h}")
                        for h in range(2, 4)
                    ]
                    dps = [
                        psD0.tile([KD, F], f32, name="dp0"),
                        psD1.tile([KD, F], f32, name="dp1"),
                        psD2.tile([KD, F], f32, name="dp2"),
                        psD3.tile([KD, F], f32, name="dp3"),
                    ]
                    for g in range(4):
                        _rec(f"D{g}({it})", nc.tensor.matmul(
                            dps[g][0:KD, :],
                            wk[32 * g : 32 * g + 20, WD0:WD1],
                            uv_slice(off, 32 * g, 32 * g + 20),
                            start=True, stop=True,
                            tile_position=(32 * g, 0),
                        ))
                    # relu: ACT covers [0 : F+AR1], DVE the rest -- both
                    # stay under the 1704ns PE cadence, and per-quarter
                    # psum tiles keep each D->relu->D WAR loop short
                    _rec(f"reluA0({it})", nc.scalar.activation(
                        ktq[0][0:KD, :], dps[0][0:KD, :], AF.Relu
                    ))
                    # last supertile: ACT takes all of q1 (it has drain
                    # slack; the DVE V1 piece otherwise runs last and
                    # stalls A1)
                    a1 = F if it == NST - 1 else AR1
                    _rec(f"reluA1({it})", nc.scalar.activation(
                        ktq[1][0:KD, 0:a1], dps[1][0:KD, 0:a1], AF.Relu
                    ))
                    if a1 < F:
                        _rec(f"reluV1({it})", nc.vector.tensor_scalar(
                            ktq[1][0:KD, a1:F], dps[1][0:KD, a1:F],
                            0.0, None, op0=ALU.max,
                        ))
                    _rec(f"reluV2({it})", nc.vector.tensor_scalar(
                        ktq[2][0:KD, :], dps[2][0:KD, :],
                        0.0, None, op0=ALU.max,
                    ))
                    _rec(f"reluV3({it})", nc.vector.tensor_scalar(
                        ktq[3][0:KD, :], dps[3][0:KD, :],
                        0.0, None, op0=ALU.max,
                    ))
                    kts[it] = ktq
                # ---- A(it-1): 4 col-tiled f16 matmuls, band q <- quarter q
                sA = it - 1
                if 0 <= sA < NST:
                    ktq = kts.pop(sA)
                    zz = psZ.tile([128, F], f32)
                    # order 0,2,3,1: the DVE relu pieces complete V2,V3,V1
                    # (scheduler frees single-buffered q2/q3 first), and PE
                    # streams execute in order -- A1 last avoids a drain
                    # stall behind its late quarter
                    for g in (0, 2, 3, 1):
                        _rec(f"A{g}({sA})", nc.tensor.matmul(
                            zz[32 * g : 32 * g + 32, :],
                            wk[0:KD, WA0:WA1],
                            ktq[g][0:KD, :],
                            start=True, stop=True,
                            tile_position=(0, 32 * g),
                        ))
                    eps[sA] = zz
                # ---- Exp(it-2), then DMA the e values straight out
                sE = it - 2
                if 0 <= sE < NST:
                    zz = eps.pop(sE)
                    ep = ep_p.tile([128, F], f16)
                    _rec(f"Exp({sE})", nc.scalar.activation(
                        ep[:, :], zz[:, :], AF.Exp
                    ))
                    oc0 = F * sE
                    nc.sync.dma_start(y_d[:, oc0 : oc0 + F], ep[:, :])
    nc.compile()
    return nc


_NC_CACHE = None


def kernel(p1, p2, W1, W2):
    global _NC_CACHE
    from concourse.bass_utils import run_bass_kernel_spmd

    P1n = _soft_perm_np(np.asarray(W1))
    P2n = _soft_perm_np(np.asarray(W2))
    wk16 = _build_wk16()
    p1 = np.asarray(p1, dtype=np.float32)
    p2 = np.asarray(p2, dtype=np.float32)
    u = np.log1p(-(p1 @ P1n.T)).astype(np.float32)
    v = np.log1p(-(p2 @ P2n.T)).astype(np.float32)

    in_maps = []
    c0 = IN_CHUNKS[0]
    for c in range(NCORES):
        sl = slice(c * BC, (c + 1) * BC)
        uvp = _build_uv(u[sl], v[sl])
        head = np.concatenate([wk16, uvp[:, :c0]], axis=1)
        in_maps.append(
            {"head": np.ascontiguousarray(head), "uvp": uvp[:, c0:].copy()}
        )

    if _NC_CACHE is None:
        _NC_CACHE = build_bass()
    res = run_bass_kernel_spmd(_NC_CACHE, in_maps, core_ids=list(range(NCORES)))
    out = np.concatenate(
        [_unpack_y(res.results[c]["yraw"]) for c in range(NCORES)], axis=0
    )
    return out
